# revision 5
# baseline (speedup 1.0000x reference)
"""TRN2 Bass kernel for nn_AttentionBlock (GroupNorm32 + 8-head attention + proj + residual).

Sharding: data-parallel over batch — batch=8, one batch element per NeuronCore, no
collectives.

Per core (v2 — rebuilt head/tail around the measured engine profile):
- Input DMAs issue first, split across the two HWDGE queues (sync + scalar) so x
  lands while weights stream in parallel.
- GroupNorm stats per 128-channel chunk: sum-of-squares on ACT (Square+accum_out),
  plain sum on DVE (tensor_reduce), group fold via tiny gmask matmuls, rsqrt as
  exp(-0.5*ln v) (the one ACT table set also serves the score exps), xhat on DVE
  tensor_scalar. qkv for the first head pair runs k-outer interleaved with the
  per-chunk stats so the PE starts real work as chunks land; a small junk-matmul
  stream covers the remaining gaps to keep the HAM clock warm (PE re-throttles to
  1.2 GHz after ~3.4us idle).
- Attention pairs as before: packed score matmuls, one (128,1024) exp per
  (sc, head), ones-column in vT so the softmax denominator falls out of the attn
  matmul. Reciprocals now run on the DVE (InstReciprocal) instead of ACT ln/exp,
  so the ACT stream during attention is exps only. Later pairs' q/k matmuls are
  deferred into earlier pairs' loops as PE filler (attention is ACT-exp-gated).
- Proj tail: attention PSUM pools closed, all 8 banks become k-outer proj
  accumulators; the final pair's division (DVE recip + gpsimd broadcast + DVE mul)
  overlaps the first k chunks; residual adds and output DMAs pipeline per tile
  across both DMA queues.

Numerics: all matmuls bf16 with fp32 PSUM accumulation; everything else fp32.

Self-contained: hardcodes shapes from the problem spec (x (8,512,32,32) f32 etc).
"""
import numpy as np
import ml_dtypes

B, CH, HH, WW = 8, 512, 32, 32
L = HH * WW                  # 1024
HEADS = 8
GROUPS = 32
EPS = 1e-5
DH = CH // HEADS             # 64
KC = CH // 128               # 4 c-chunks
OC3 = 3 * CH // 128          # 12 qkv o-chunks
SC = L // 128                # 8 s/l-chunks
TC = L // 512                # 2 t-chunks
GN_N = (CH // GROUPS) * L    # elements per group = 16384

_cache = {}


def _build(has_qkv_bias, has_proj_bias):
    import concourse.bass as bass
    import concourse.tile as tile
    from concourse import bacc, mybir
    import bass_rust as _bass_rust
    from concourse.hw_specs import get_activation_tables

    F32 = mybir.dt.float32
    BF16 = mybir.dt.bfloat16
    AF = mybir.ActivationFunctionType
    OP = mybir.AluOpType
    AX = mybir.AxisListType

    class _Bacc(bacc.Bacc):
        # Pin Exp/Ln to the combined `natural_log_exp_and_others` table set so
        # alternating Ln/Exp activations don't thrash ACT_TABLE_LOADs (~2.7us
        # each). Same algorithm as Bacc.insert_act_table_loads, with Exp/Ln
        # stripped from every other set so the chooser can't pick them.
        def insert_act_table_loads(self):
            has_activation = any(
                isinstance(i, mybir.InstActivation)
                for b in self.main_func.blocks
                for i in b.instructions
            )
            if not has_activation:
                return
            combo = {AF.Exp, AF.Ln}
            tables = []
            for name, fns in get_activation_tables(self.m.arch).items():
                if name != "natural_log_exp_and_others":
                    fns = {f for f in fns if f not in combo}
                tables.append((name, fns))
            _bass_rust.insert_act_table_loads(self, tables)

    nc = _Bacc("TRN2", target_bir_lowering=False, debug=False, num_devices=8)

    x_d = nc.dram_tensor("x", [CH, L], F32, kind="ExternalInput").ap()
    qw_d = nc.dram_tensor("qkv_wt", [CH, 3 * CH], BF16, kind="ExternalInput").ap()
    pw_d = nc.dram_tensor("proj_wt", [CH, CH], BF16, kind="ExternalInput").ap()
    gmask_d = nc.dram_tensor("gmask", [128, 8], F32, kind="ExternalInput").ap()
    gmaskT_d = nc.dram_tensor("gmask_t", [8, 128], F32, kind="ExternalInput").ap()
    if has_qkv_bias:
        qkb_d = nc.dram_tensor("qk_bias", [128, 8], F32, kind="ExternalInput").ap()
        vb_d = nc.dram_tensor("v_bias", [128, KC], F32, kind="ExternalInput").ap()
    if has_proj_bias:
        pb_d = nc.dram_tensor("p_bias", [128, KC], F32, kind="ExternalInput").ap()
    out_d = nc.dram_tensor("out", [CH, L], F32, kind="ExternalOutput").ap()

    with tile.TileContext(nc) as tc:
        import contextlib
        ctx = contextlib.ExitStack()
        pers = ctx.enter_context(tc.tile_pool(name="pers", bufs=1))
        scr = ctx.enter_context(tc.tile_pool(name="scr", bufs=2))
        ewp = ctx.enter_context(tc.tile_pool(name="ewp", bufs=8))
        dvp = ctx.enter_context(tc.tile_pool(name="dvp", bufs=2))
        asg = ctx.enter_context(tc.tile_pool(name="asg", bufs=8))
        outp = ctx.enter_context(tc.tile_pool(name="outp", bufs=3))

        # ---- input DMAs first: x on the sync queue, weights/masks on the
        # scalar (ACT) queue so both HWDGE queues stream concurrently ----
        xs = pers.tile([128, KC * L], F32, tag="xs")
        for k in range(KC):
            nc.sync.dma_start(xs[:, k * L:(k + 1) * L], x_d[128 * k:128 * (k + 1), :])
        gmask = pers.tile([128, 8], F32, tag="gmask")
        nc.scalar.dma_start(gmask[:], gmask_d[:])
        gmaskT = pers.tile([8, 128], F32, tag="gmask_t")
        nc.scalar.dma_start(gmaskT[:], gmaskT_d[:])
        if has_qkv_bias:
            qkb = pers.tile([128, 8], F32, tag="qkb")
            nc.scalar.dma_start(qkb[:], qkb_d[:])
            vb = pers.tile([128, KC], F32, tag="vb")
            nc.scalar.dma_start(vb[:], vb_d[:])
        if has_proj_bias:
            pb = pers.tile([128, KC], F32, tag="pb")
            nc.scalar.dma_start(pb[:], pb_d[:])
        qw = pers.tile([128, KC * 3 * CH], BF16, tag="qw")
        for k in range(KC):
            nc.scalar.dma_start(qw[:, k * 3 * CH:(k + 1) * 3 * CH],
                                qw_d[128 * k:128 * (k + 1), :])
        pw = pers.tile([128, KC * CH], BF16, tag="pw")
        for k in range(KC):
            nc.scalar.dma_start(pw[:, k * CH:(k + 1) * CH], pw_d[128 * k:128 * (k + 1), :])

        # ---- small constants + ACT table-load trigger ----
        epsb = pers.tile([8, 1], F32, tag="epsb")
        nc.gpsimd.memset(epsb[:], EPS)
        tldt = pers.tile([8, 1], F32, tag="tldt")
        nc.scalar.activation(tldt[:], epsb[:], AF.Exp)

        # ---- junk-matmul stream: keeps the PE HAM clock warm through the
        # startup DMA/stats latency (in-order PE would otherwise idle) ----
        wsrc = pers.tile([128, 640], BF16, tag="wsrc")
        nc.gpsimd.memset(wsrc[:], 0.0)
        head_psum = tc.tile_pool(name="psH", bufs=1, space="PSUM")
        psH = head_psum.__enter__()
        jps = psH.tile([128, 512], F32, tag="junk")

        def junk(n):
            for _ in range(n):
                nc.tensor.matmul(jps[:], wsrc[:, 0:128], wsrc[:, 128:640],
                                 start=True, stop=True)

        junk(10)

        # ---- GroupNorm stats + xhat, per chunk; qkv for pair 0 (o-chunks
        # j=0 q / j=4 k) interleaved k-outer so PE consumes chunks as the
        # stats pipeline produces them ----
        stat = pers.tile([128, 8], F32, tag="stat")  # cols 2k: sum(x), 2k+1: sum(x^2)
        xhat = pers.tile([128, KC * L], BF16, tag="xhat")
        bc = pers.tile([128, 2 * KC], F32, tag="bc")  # cols 2k mean, 2k+1 rstd
        qk = pers.tile([128, 8 * L], BF16, tag="qk")   # o-chunk j: j=0-3 q, 4-7 k

        psk = {}
        for j in (0, 4):
            for t in range(TC):
                psk[(j, t)] = psH.tile([128, 512], F32, tag="qkps",
                                       name=f"psk{j}_{t}")

        for k in range(KC):
            xk = xs[:, k * L:(k + 1) * L]
            sq = scr.tile([128, L], F32, tag="sq")
            nc.scalar.activation(sq[:], xk, AF.Square,
                                 accum_out=stat[:, 2 * k + 1:2 * k + 2])
            nc.vector.tensor_reduce(stat[:, 2 * k:2 * k + 1], xk,
                                    axis=AX.X, op=OP.add)
            gst_ps = psH.tile([8, 2], F32, tag="gst")
            nc.tensor.matmul(gst_ps[:], gmask[:], stat[:, 2 * k:2 * k + 2],
                             start=True, stop=True)
            s2k = pers.tile([8, 2], F32, tag=f"s2k{k}")   # col 0 mean, col 1 rstd
            vk = pers.tile([8, 2], F32, tag=f"vk{k}")     # col 0 var, col 1 scratch
            nc.vector.tensor_scalar_mul(s2k[:], gst_ps[:], 1.0 / GN_N)  # mean, E[x^2]
            nc.vector.tensor_mul(vk[:, 1:2], s2k[:, 0:1], s2k[:, 0:1])  # mean^2
            nc.vector.tensor_sub(vk[:, 0:1], s2k[:, 1:2], vk[:, 1:2])   # var
            nc.scalar.activation(vk[:, 1:2], vk[:, 0:1], AF.Ln, bias=epsb[:])
            nc.scalar.activation(s2k[:, 1:2], vk[:, 1:2], AF.Exp, scale=-0.5)
            bc_ps = psH.tile([128, 2], F32, tag="bcps")
            nc.tensor.matmul(bc_ps[:], gmaskT[:], s2k[:], start=True, stop=True)
            nc.vector.tensor_copy(bc[:, 2 * k:2 * k + 2], bc_ps[:])
            nmr = pers.tile([128, 1], F32, tag=f"nmr{k}")   # -mean*rstd
            nc.vector.tensor_scalar(
                out=nmr[:], in0=bc[:, 2 * k:2 * k + 1],
                scalar1=bc[:, 2 * k + 1:2 * k + 2], scalar2=-1.0,
                op0=OP.mult, op1=OP.mult)
            nc.vector.tensor_scalar(
                out=xhat[:, k * L:(k + 1) * L], in0=xk,
                scalar1=bc[:, 2 * k + 1:2 * k + 2], scalar2=nmr[:],
                op0=OP.mult, op1=OP.add)
            # pair-0 q/k matmuls for this chunk (LDW shared across t)
            for j in (0, 4):
                for t in range(TC):
                    nc.tensor.matmul(
                        psk[(j, t)][:],
                        qw[:, k * 3 * CH + 128 * j:k * 3 * CH + 128 * (j + 1)],
                        xhat[:, k * L + 512 * t:k * L + 512 * (t + 1)],
                        start=(k == 0), stop=(k == KC - 1))
            junk(3)

        def qk_store(j, t, ps_ap):
            dst = qk[:, j * L + 512 * t:j * L + 512 * (t + 1)]
            if has_qkv_bias:
                nc.vector.tensor_scalar_add(dst, ps_ap, qkb[:, j:j + 1])
            else:
                nc.vector.tensor_copy(dst, ps_ap)

        for j in (0, 4):
            for t in range(TC):
                qk_store(j, t, psk[(j, t)][:])

        # ---- vT with appended ones-column (softmax denominator rides the
        # attention matmul); all xhat chunks are ready here so this is dense ----
        vt = pers.tile([128, SC * (HEADS * 65)], BF16, tag="vt")
        for lc in range(SC):
            v3 = vt[:, lc * 520:(lc + 1) * 520].rearrange("p (h c) -> p h c", c=65)
            nc.gpsimd.memset(v3[:, :, 64:65], 1.0)
        for lc in range(SC):
            ps = psH.tile([128, 512], F32, tag="qkps")
            for k in range(KC):
                nc.tensor.matmul(
                    ps[:], xhat[:, k * L + 128 * lc:k * L + 128 * (lc + 1)],
                    qw[:, k * 3 * CH + 2 * CH:k * 3 * CH + 3 * CH],
                    start=(k == 0), stop=(k == KC - 1))
            v3 = vt[:, lc * 520:(lc + 1) * 520].rearrange("p (h c) -> p h c", c=65)
            src = ps[:].rearrange("p (h c) -> p h c", c=64)
            nc.vector.tensor_copy(v3[:, :, 0:64], src)

        # ---- pair-1 q/k (j=1,5), k-outer for LDW pairing ----
        psk2 = {}
        for j in (1, 5):
            for t in range(TC):
                psk2[(j, t)] = psH.tile([128, 512], F32, tag="qkps",
                                        name=f"psk{j}_{t}")
        for k in range(KC):
            for j in (1, 5):
                for t in range(TC):
                    nc.tensor.matmul(
                        psk2[(j, t)][:],
                        qw[:, k * 3 * CH + 128 * j:k * 3 * CH + 128 * (j + 1)],
                        xhat[:, k * L + 512 * t:k * L + 512 * (t + 1)],
                        start=(k == 0), stop=(k == KC - 1))
        for j in (1, 5):
            for t in range(TC):
                qk_store(j, t, psk2[(j, t)][:])
        head_psum.__exit__(None, None, None)

        # deferred q/k emission for pairs 2-3 (PE filler inside the
        # ACT-exp-gated attention steady state)
        def emit_qk(j, pool, width):
            for t in range(TC):
                ps = pool.tile([128, width], F32, tag="ps")
                for k in range(KC):
                    nc.tensor.matmul(
                        ps[:, 0:512],
                        qw[:, k * 3 * CH + 128 * j:k * 3 * CH + 128 * (j + 1)],
                        xhat[:, k * L + 512 * t:k * L + 512 * (t + 1)],
                        start=(k == 0), stop=(k == KC - 1))
                qk_store(j, t, ps[:, 0:512])

        # ---- attention, head pairs (2m, 2m+1) packed into PE row groups ----
        a_sb = pers.tile([128, KC * L], BF16, tag="a_sb")
        attn_psum = tc.tile_pool(name="psS", bufs=2, space="PSUM")
        psS = attn_psum.__enter__()
        attn_acc = tc.tile_pool(name="psA", bufs=4, space="PSUM")
        psA = attn_acc.__enter__()

        def div_recip(stgs):
            # Fold the four 512-wide ones-row sums into (128,16) via tiny
            # SBUF->SBUF DMAs (DMA engines are idle here), reciprocal on DVE,
            # then unfold back to a partition-0 row for the gpsimd broadcast.
            den128 = dvp.tile([128, 16], F32, tag="d128")
            for i, (sg, e, t, mm_) in enumerate(stgs):
                nc.sync.dma_start(den128[:, 4 * i:4 * (i + 1)], sg[64:65, :])
            r128 = dvp.tile([128, 16], F32, tag="r128")
            nc.vector.reciprocal(r128[:], den128[:])
            rden = dvp.tile([1, 4 * 512], F32, tag="rden")
            for i in range(4):
                nc.sync.dma_start(rden[0:1, 512 * i:512 * (i + 1)],
                                  r128[:, 4 * i:4 * (i + 1)])
            return rden

        def div_mul(rden, i, sg, e, t, mm_):
            bsb = dvp.tile([64, 512], F32, tag="bsb")
            nc.gpsimd.partition_broadcast(bsb[:], rden[0:1, 512 * i:512 * (i + 1)])
            dst = a_sb[64 * e:64 * (e + 1),
                       mm_ * L + 512 * t:mm_ * L + 512 * (t + 1)]
            nc.vector.tensor_mul(dst, sg[0:64, :], bsb[:])
            if has_qkv_bias:
                nc.vector.tensor_scalar_add(
                    dst, dst, vb[64 * e:64 * (e + 1), mm_:mm_ + 1])

        def division_steps(stgs):
            # generator: one cheap step per scheduling slot
            rden = div_recip(stgs)
            yield
            for i, (sg, e, t, mm_) in enumerate(stgs):
                div_mul(rden, i, sg, e, t, mm_)
                if i % 2 == 1:
                    yield

        pending_div = None
        for m in range(4):
            ps_a = [[None, None], [None, None]]
            for e in range(2):
                for t in range(TC):
                    pa = psA.tile([65, 512], F32, tag="pa")
                    ps_a[e][t] = pa

            def q_ap(e, t):
                return qk[64 * e:64 * (e + 1), m * L + 512 * t:m * L + 512 * (t + 1)]

            def k_ap(e, sc):
                return qk[64 * e:64 * (e + 1),
                          (4 + m) * L + 128 * sc:(4 + m) * L + 128 * (sc + 1)]

            def attn_mm(sc, e):
                ew = ew_tiles[(sc, e)]
                for t in range(TC):
                    nc.tensor.matmul(
                        ps_a[e][t][:],
                        vt[:, sc * 520 + (2 * m + e) * 65:
                           sc * 520 + (2 * m + e) * 65 + 65],
                        ew[:, 512 * t:512 * (t + 1)],
                        start=(sc == 0), stop=(sc == SC - 1))

            ew_tiles = {}
            for sc in range(SC):
                ps_w = [None, None]
                for e in range(2):
                    pw_t = psS.tile([128, 1024], F32, tag="ps")
                    ps_w[e] = pw_t
                # packed score MM pairs (head 2m rows 0-63, head 2m+1 rows 64-127)
                for t in range(TC):
                    for e in range(2):
                        nc.tensor.matmul(ps_w[e][:, 512 * t:512 * (t + 1)],
                                         k_ap(e, sc), q_ap(e, t),
                                         start=True, stop=True)
                for e in range(2):
                    ew = ewp.tile([128, L], BF16, tag="ew")
                    ew_tiles[(sc, e)] = ew
                    nc.scalar.activation(ew[:], ps_w[e][:], AF.Exp)
                # previous pair's division, one step per sc to spread the load
                if pending_div is not None:
                    next(pending_div, None)
                # deferred q/k matmuls for pair m+2 act as PE filler in the
                # ACT-bound attention steady state
                if m < 2 and sc == 2:
                    emit_qk(m + 2, psS, 1024)
                if m < 2 and sc == 5:
                    emit_qk(4 + m + 2, psS, 1024)
                # software-pipeline: attn MMs for sc-1 after scores for sc
                if sc > 0:
                    for e in range(2):
                        attn_mm(sc - 1, e)
            for e in range(2):
                attn_mm(SC - 1, e)

            # stage accumulators to SBUF so the PSUM banks free up for the
            # next head pair; the divisions run interleaved with the NEXT
            # pair's exp stream (pending_div) to avoid a DVE lump here.
            if pending_div is not None:
                for _ in pending_div:  # flush any leftovers of pair m-1
                    pass
            stgs = []
            for e in range(2):
                for t in range(TC):
                    sg = asg.tile([65, 512], F32, tag="astg")
                    nc.vector.tensor_copy(sg[:], ps_a[e][t][:])
                    stgs.append((sg, e, t, m))
            if m < 3:
                pending_div = division_steps(stgs)
            else:
                pending_div = None
                final_stgs = stgs
        attn_acc.__exit__(None, None, None)
        attn_psum.__exit__(None, None, None)

        # ---- proj + residual: all 8 PSUM banks as k-outer accumulators; the
        # final pair's division overlaps the k=0..2 waves; adds + out-DMAs
        # pipeline per tile across both DMA queues ----
        with tc.tile_pool(name="psP", bufs=8, space="PSUM") as psP:
            pstiles = {}
            for t in range(TC):
                for i in range(KC):
                    pstiles[(t, i)] = psP.tile([128, 512], F32, tag="ps",
                                               name=f"psp{t}_{i}")
            for k in range(KC):
                if k == 0:
                    final_rden = div_recip(final_stgs)
                for i in range(KC):
                    for t in range(TC):
                        nc.tensor.matmul(
                            pstiles[(t, i)][:],
                            pw[:, k * CH + 128 * i:k * CH + 128 * (i + 1)],
                            a_sb[:, k * L + 512 * t:k * L + 512 * (t + 1)],
                            start=(k == 0), stop=(k == KC - 1))
                if k == 0:
                    for i_, (sg, e, tt, mm_) in enumerate(final_stgs):
                        div_mul(final_rden, i_, sg, e, tt, mm_)
            for i in range(KC):
                for t in range(TC):
                    ot = outp.tile([128, 512], F32, tag="ot")
                    nc.vector.tensor_add(ot[:],
                                         xs[:, i * L + 512 * t:i * L + 512 * (t + 1)],
                                         pstiles[(t, i)][:])
                    if has_proj_bias:
                        nc.vector.tensor_scalar_add(ot[:], ot[:], pb[:, i:i + 1])
                    eng = nc.sync if (2 * i + t) % 2 == 0 else nc.scalar
                    eng.dma_start(
                        out_d[128 * i:128 * (i + 1), 512 * t:512 * (t + 1)], ot[:])
        ctx.close()

    nc.compile()
    return nc


def _prep_inputs(x, norm_w, norm_b, qkv_w, qkv_b, proj_w, proj_b):
    scale = DH ** -0.25
    w_eff = (qkv_w.astype(np.float64) * norm_w.astype(np.float64)[None, :])
    b_eff = qkv_b.astype(np.float64) + w_eff @ norm_b.astype(np.float64)
    # reference splits qkv per head: row h*192 + {0:64 q, 64:128 k, 128:192 v}.
    # device layout wants [q_all_heads | k_all_heads | v_all_heads], head-major.
    perm = np.concatenate([
        np.concatenate([np.arange(h * 3 * DH + t * DH, h * 3 * DH + (t + 1) * DH)
                        for h in range(HEADS)])
        for t in range(3)])
    w_eff = w_eff[perm]
    b_eff = b_eff[perm]
    w_eff[:2 * CH] *= scale
    b_eff[:2 * CH] *= scale
    qkv_wt = np.ascontiguousarray(w_eff.T).astype(np.float32).astype(ml_dtypes.bfloat16)
    proj_wt = np.ascontiguousarray(proj_w.T).astype(ml_dtypes.bfloat16)

    p = np.arange(128)
    gmask = (p[:, None] // 16 == np.arange(8)[None, :]).astype(np.float32)
    gmask_t = np.ascontiguousarray(gmask.T)

    has_qkv_bias = bool(np.any(b_eff != 0.0))
    has_proj_bias = bool(np.any(proj_b != 0.0))
    common = {"qkv_wt": qkv_wt, "proj_wt": proj_wt, "gmask": gmask,
              "gmask_t": gmask_t}
    if has_qkv_bias:
        qk_part = b_eff[:2 * CH].astype(np.float32).reshape(8, 128).T
        v_part = b_eff[2 * CH:].astype(np.float32).reshape(KC, 128).T
        common["qk_bias"] = np.ascontiguousarray(qk_part)
        common["v_bias"] = np.ascontiguousarray(v_part)
    if has_proj_bias:
        common["p_bias"] = np.ascontiguousarray(
            proj_b.astype(np.float32).reshape(KC, 128).T)
    xf = np.ascontiguousarray(x.reshape(B, CH, L)).astype(np.float32)
    in_maps = [dict(common, x=np.ascontiguousarray(xf[i])) for i in range(B)]
    return in_maps, has_qkv_bias, has_proj_bias


def _get_nc(flags):
    if flags not in _cache:
        _cache[flags] = _build(*flags)
    return _cache[flags]


def _run(inputs, trace=False, tmpdir=None):
    import time
    from concourse.bass_utils import run_bass_kernel_spmd
    in_maps, hqb, hpb = _prep_inputs(**inputs)
    nc = _get_nc((hqb, hpb))
    kw = {}
    if trace:
        kw = dict(trace=True, tmpdir=tmpdir)
    last_err = None
    for attempt in range(3):
        # the very first execution on a freshly-attached device occasionally
        # fails with NRT_EXEC_UNIT_UNRECOVERABLE; a retry recovers it
        try:
            res = run_bass_kernel_spmd(nc, in_maps, list(range(B)), **kw)
            break
        except Exception as e:  # noqa: BLE001
            last_err = e
            time.sleep(5)
    else:
        raise last_err
    out = np.stack([res.results[i]["out"] for i in range(B)])
    return out.reshape(B, CH, HH, WW).astype(np.float32), res


def kernel(x, norm_w, norm_b, qkv_w, qkv_b, proj_w, proj_b):
    out, _ = _run(dict(x=x, norm_w=norm_w, norm_b=norm_b, qkv_w=qkv_w,
                       qkv_b=qkv_b, proj_w=proj_w, proj_b=proj_b))
    return out
